# revision 11
# baseline (speedup 1.0000x reference)
"""NT-Xent contrastive loss on 8 Trainium2 NeuronCores (V3.1, bf16 + XBAR).

Math (reference): z = [z_i; z_j] (N=8192, D=128), zn = z/||z||,
sim = zn@zn.T / 0.1.  Row loss_i = logsumexp_{j!=i} sim[i,j] - sim[i, pos(i)],
loss = mean_i loss_i.

Sharding: rolled-column trick.  Core c receives z rolled by -1024*c rows.
Its 1024 local rows are rolled rows 0..1023; in rolled coordinates the
self column of local row i is i and the positive column is i + 4096 on
EVERY core, so a single static SPMD program works with no collectives.
The self logit is suppressed by adding -5 to the diagonal cosine
(logit -40 -> exp ~4e-18, negligible).  Host sums the 8 partial means.

V3.1 schedule:
  - znT bf16 via XBAR DMA-transpose (SP queue), per 512-row group: the
    first column chunk's four 512-wide matmuls start as each group lands.
  - PSUM = 2 x [128,2048] double buffer (all 8 banks) for sim matmuls;
    ACT exps each chunk in place with accum_out row sums.
  - Stage-A (square/reduce/scale) runs in 512-row groups, one ~1.1us DVE
    insert per chunk so the diag-shift add never queues behind big work.
  - pos_i = zn_i . zn_{i+4096} is computed from zin tiles + inv norms at
    q=3 (decoupled from the PSUM sim chunks entirely).
"""

import os
import sys

import numpy as np

_TRN_REPO = "/opt/trn_rl_repo"
if _TRN_REPO not in sys.path:
    sys.path.insert(0, _TRN_REPO)

from concourse import bacc, bass, mybir, tile
from concourse.bass_utils import run_bass_kernel_spmd

B = 4096
D = 128
N = 2 * B
N_CORES = 8
RPC = N // N_CORES  # 1024 rows per core
INV_T = 10.0
DIAG_SHIFT = -5.0

NBATCH = 4  # stage-A batches of 2048 rows
TPB = 16    # 128-row tiles per batch
NGRP = 16   # 512-row groups
TPG = 4     # tiles per group
RB = 8      # row blocks per core (128 rows each)
QB = 4      # 2048-wide column chunks
KB = 4      # 512-wide matmuls per chunk

_cache: dict = {}


def build():
    f32 = mybir.dt.float32
    bf16 = mybir.dt.bfloat16
    AX = mybir.AxisListType
    AF = mybir.ActivationFunctionType

    nc = bacc.Bacc(
        "TRN2", target_bir_lowering=False, debug=False, num_devices=N_CORES
    )

    # Pin ln/exp/copy/etc to one ACT table: avoids 1.3us ACT_TABLE_LOAD at
    # every ln<->exp transition.
    tabs = bacc.get_activation_tables(nc.m.arch)
    pinned = set(tabs["natural_log_exp_and_others"])
    for k in tabs:
        if k != "natural_log_exp_and_others":
            tabs[k] = tabs[k] - pinned

    z_dram = nc.dram_tensor("z_roll", [N, D], f32, kind="ExternalInput")
    loss_dram = nc.dram_tensor("loss_part", [1, 1], f32, kind="ExternalOutput")

    eye_np = np.eye(128, dtype=np.float32)
    negI_dram = nc.inline_tensor(
        (DIAG_SHIFT * eye_np).astype(np.float32), name="negI128"
    )
    ones_dram = nc.inline_tensor(np.ones((128, 1), np.float32), name="ones128")

    with tile.TileContext(nc) as tc:
        with (
            tc.tile_pool(name="const", bufs=1) as cpool,
            tc.tile_pool(name="zin", bufs=NBATCH) as zpool,
            tc.tile_pool(name="zn", bufs=4) as npool,
            tc.tile_pool(name="persist", bufs=1) as ppool,
            tc.tile_pool(name="scr", bufs=2) as spool,
            tc.tile_pool(name="pscr", bufs=2) as pspool,
            tc.tile_pool(name="psum", bufs=2, space=bass.MemorySpace.PSUM) as qpool,
        ):
            negI_sb = cpool.tile([128, 128], f32)
            ones_sb = cpool.tile([128, 1], f32)

            ssq = ppool.tile([128, NBATCH * TPB], f32)
            lnssq = ppool.tile([128, NBATCH * TPB], f32)
            inv = ppool.tile([128, NBATCH * TPB], f32)
            znT = ppool.tile([128, N], bf16)
            # accum slots: 0..3 = q0 512-wide warmup pieces (r=0 only),
            # 4..6 = chunks q=1..3; unused slots are memset to 0.
            sexp = ppool.tile([128, RB, QB + 3], f32)
            pos = ppool.tile([128, RB], f32)

            # Input DMAs: batch 0's four sub-DMAs go first on the idle SP
            # queue (565ns issue each); batches 1-3 + consts on gpsimd.
            zin_tiles = [
                zpool.tile([128, TPB, 128], f32, name=f"zin{b}")
                for b in range(NBATCH)
            ]
            for b in range(NBATCH):
                eng = nc.sync if b == 0 else nc.gpsimd
                for s in range(4):
                    r0 = 2048 * b + 512 * s
                    src = z_dram[r0 : r0 + 512, :].rearrange(
                        "(t p) d -> p t d", p=128
                    )
                    eng.dma_start(zin_tiles[b][:, 4 * s : 4 * s + 4, :], src)
            nc.gpsimd.dma_start(negI_sb[:], negI_dram[:])
            nc.gpsimd.dma_start(ones_sb[:], ones_dram[:])

            scr_g = {}

            def ssq_mul(g):
                b, s = g // 4, g % 4
                scr = spool.tile([128, TPG * 128], f32, tag="sq")
                scr_g[g] = scr
                zv = zin_tiles[b][:, 4 * s : 4 * s + 4, :].rearrange(
                    "p t d -> p (t d)"
                )
                nc.vector.tensor_mul(scr[:], zv, zv)

            def ssq_red(g):
                b, s = g // 4, g % 4
                j0 = TPB * b + TPG * s
                nc.vector.reduce_sum(
                    ssq[:, j0 : j0 + TPG],
                    scr_g.pop(g)[:].rearrange("p (t d) -> p t d", d=128),
                    axis=AX.X,
                )

            def norms(b, g0=None):
                # 1/||z|| = exp(-0.5*ln(ssq)); stays in the Ln/Exp ACT table.
                # g0 set: per-group [128,4] norms (batch-0 pipelining).
                if g0 is None:
                    j0, w = TPB * b, TPB
                else:
                    j0, w = TPG * g0, TPG
                nc.scalar.activation(
                    lnssq[:, j0 : j0 + w], ssq[:, j0 : j0 + w], AF.Ln
                )
                nc.scalar.activation(
                    inv[:, j0 : j0 + w], lnssq[:, j0 : j0 + w],
                    AF.Exp, scale=-0.5,
                )

            zn_tiles = {}

            def tsm(g):
                b, s = g // 4, g % 4
                zn = zn_tiles[g] = npool.tile(
                    [128, TPG, 128], bf16, name=f"zn{g}", tag="zn"
                )
                for t in range(4):
                    j = TPB * b + 4 * s + t
                    nc.vector.tensor_scalar_mul(
                        zn[:, t, :],
                        zin_tiles[b][:, 4 * s + t, :],
                        inv[:, j : j + 1],
                    )

            def build_trans(g):
                zn = zn_tiles.pop(g)
                c0 = 512 * g
                nc.sync.dma_start_transpose(
                    znT[:, c0 : c0 + 512].rearrange("p (t c) -> p t c", c=128),
                    zn[:].rearrange("p t d -> p (t d)"),
                )

            def pos_dot(r):
                # pos_r = 10 * cos(row_r, row_r+4096) from raw zin tiles:
                # dot on zin, then scale by the two inverse norms.
                scr = pspool.tile([128, 128], f32, tag="pd")
                nc.vector.tensor_mul(
                    scr[:], zin_tiles[0][:, r, :], zin_tiles[2][:, r, :]
                )
                dot = pspool.tile([128, 1], f32, tag="pe")
                nc.vector.reduce_sum(dot[:], scr[:], axis=AX.X)
                nc.vector.tensor_mul(dot[:], dot[:], inv[:, r : r + 1])
                nc.vector.tensor_mul(
                    pos[:, r : r + 1], dot[:], inv[:, 32 + r : 33 + r]
                )

            # --- prologue: batch 0 only ---
            # DVE runs the 4 muls/reds back-to-back (per-group norms overlap
            # on ACT); tsm+xbar chains follow per group.
            nc.vector.memset(sexp[:], 0.0)
            for g in range(4):
                ssq_mul(g)
                ssq_red(g)
                norms(0, g0=g)
            for g in range(4):
                tsm(g)
                build_trans(g)

            # --- main loop: q-outer, r-inner; build batch q+1 under chunk q ---
            for q in range(QB):
                b = q + 1
                for r in range(RB):
                    lhsT = znT[:, 128 * r : 128 * (r + 1)]
                    ps = qpool.tile([128, 2048], f32, tag="mm")
                    for k in range(KB):
                        c0 = 2048 * q + 512 * k
                        nc.tensor.matmul(
                            ps[:, 512 * k : 512 * (k + 1)],
                            lhsT,
                            znT[:, c0 : c0 + 512],
                            start=True,
                            stop=True,
                        )
                    if q == 0:
                        sub = ps[:, 128 * r : 128 * (r + 1)]
                        nc.vector.tensor_add(sub, sub, negI_sb[:])
                    if q == 0 and r == 0:
                        # warmup: 512-wide exps so ACT starts as soon as the
                        # first znT group lands, not after all four.
                        for k in range(KB):
                            nc.scalar.activation(
                                ps[:, 512 * k : 512 * (k + 1)],
                                ps[:, 512 * k : 512 * (k + 1)],
                                AF.Exp,
                                scale=INV_T,
                                accum_out=sexp[:, 0, k : k + 1],
                            )
                    else:
                        nc.scalar.activation(
                            ps[:],
                            ps[:],
                            AF.Exp,
                            scale=INV_T,
                            accum_out=sexp[:, r, 3 + q : 4 + q],
                        )
                    if b < NBATCH:
                        g = 4 * b + (r % 4)
                        if r < 4:
                            ssq_mul(g)
                            ssq_red(g)
                        else:
                            if r == 4:
                                norms(b)
                            tsm(g)
                            build_trans(g)
                    elif q == QB - 1:
                        pos_dot(r)

            # --- epilogue ---
            s8 = ppool.tile([128, RB], f32)
            nc.vector.reduce_sum(s8[:], sexp[:], axis=AX.X)
            lse = ppool.tile([128, RB], f32)
            nc.scalar.activation(lse[:], s8[:], AF.Ln)
            poss = ppool.tile([128, RB], f32)
            nc.scalar.mul(poss[:], pos[:], INV_T)
            acc = ppool.tile([128, RB], f32)
            nc.vector.tensor_sub(acc[:], lse[:], poss[:])
            tot = ppool.tile([128, 1], f32)
            nc.vector.reduce_sum(tot[:], acc[:], axis=AX.X)
            psf = qpool.tile([128, 2048], f32, tag="mm")
            nc.tensor.matmul(
                psf[0:1, 0:1], ones_sb[:], tot[:], start=True, stop=True
            )
            res = ppool.tile([1, 1], f32)
            nc.scalar.mul(res[:], psf[0:1, 0:1], 1.0 / N)
            nc.gpsimd.dma_start(loss_dram[:], res[:])

    nc.compile()
    return nc


def get_nc():
    if "nc" not in _cache:
        _cache["nc"] = build()
    return _cache["nc"]


def make_in_maps(z_i: np.ndarray, z_j: np.ndarray):
    z = np.concatenate(
        [np.asarray(z_i, np.float32), np.asarray(z_j, np.float32)], axis=0
    )
    return [
        {"z_roll": np.ascontiguousarray(np.roll(z, -RPC * c, axis=0))}
        for c in range(N_CORES)
    ]


def kernel(**inputs) -> np.ndarray:
    in_maps = make_in_maps(inputs["z_i"], inputs["z_j"])
    nc = get_nc()
    res = run_bass_kernel_spmd(nc, in_maps, list(range(N_CORES)))
    kernel.last_results = res
    total = np.float32(0.0)
    for r in res.results:
        total = np.float32(total + np.float32(np.asarray(r["loss_part"]).reshape(())))
    return np.float32(total)


# revision 14
# speedup vs baseline: 1.0391x; 1.0391x over previous
"""NT-Xent contrastive loss on 8 Trainium2 NeuronCores (V3.1, bf16 + XBAR).

Math (reference): z = [z_i; z_j] (N=8192, D=128), zn = z/||z||,
sim = zn@zn.T / 0.1.  Row loss_i = logsumexp_{j!=i} sim[i,j] - sim[i, pos(i)],
loss = mean_i loss_i.

Sharding: rolled-column trick.  Core c receives z rolled by -1024*c rows.
Its 1024 local rows are rolled rows 0..1023; in rolled coordinates the
self column of local row i is i and the positive column is i + 4096 on
EVERY core, so a single static SPMD program works with no collectives.
The self logit is suppressed by adding -5 to the diagonal cosine
(logit -40 -> exp ~4e-18, negligible).  Host sums the 8 partial means.

V3.1 schedule:
  - znT bf16 via XBAR DMA-transpose (SP queue), per 512-row group: the
    first column chunk's four 512-wide matmuls start as each group lands.
  - PSUM = 2 x [128,2048] double buffer (all 8 banks) for sim matmuls;
    ACT exps each chunk in place with accum_out row sums.
  - Stage-A (square/reduce/scale) runs in 512-row groups, one ~1.1us DVE
    insert per chunk so the diag-shift add never queues behind big work.
  - pos_i = zn_i . zn_{i+4096} is computed from zin tiles + inv norms at
    q=3 (decoupled from the PSUM sim chunks entirely).
"""

import os
import sys

import numpy as np

_TRN_REPO = "/opt/trn_rl_repo"
if _TRN_REPO not in sys.path:
    sys.path.insert(0, _TRN_REPO)

from concourse import bacc, bass, mybir, tile
from concourse.bass_utils import run_bass_kernel_spmd

B = 4096
D = 128
N = 2 * B
N_CORES = 8
RPC = N // N_CORES  # 1024 rows per core
INV_T = 10.0
DIAG_SHIFT = -5.0

NBATCH = 4  # stage-A batches of 2048 rows
TPB = 16    # 128-row tiles per batch
NGRP = 16   # 512-row groups
TPG = 4     # tiles per group
RB = 8      # row blocks per core (128 rows each)
QB = 4      # 2048-wide column chunks
KB = 4      # 512-wide matmuls per chunk

_cache: dict = {}


def build():
    f32 = mybir.dt.float32
    bf16 = mybir.dt.bfloat16
    AX = mybir.AxisListType
    AF = mybir.ActivationFunctionType

    nc = bacc.Bacc(
        "TRN2", target_bir_lowering=False, debug=False, num_devices=N_CORES
    )

    # Pin ln/exp/copy/etc to one ACT table: avoids 1.3us ACT_TABLE_LOAD at
    # every ln<->exp transition.
    tabs = bacc.get_activation_tables(nc.m.arch)
    pinned = set(tabs["natural_log_exp_and_others"])
    for k in tabs:
        if k != "natural_log_exp_and_others":
            tabs[k] = tabs[k] - pinned

    z_dram = nc.dram_tensor("z_roll", [N, D], f32, kind="ExternalInput")
    loss_dram = nc.dram_tensor("loss_part", [1, 1], f32, kind="ExternalOutput")

    eye_np = np.eye(128, dtype=np.float32)
    negI_dram = nc.inline_tensor(
        (DIAG_SHIFT * eye_np).astype(np.float32), name="negI128"
    )
    ones_dram = nc.inline_tensor(np.ones((128, 1), np.float32), name="ones128")

    with tile.TileContext(nc) as tc:
        with (
            tc.tile_pool(name="const", bufs=1) as cpool,
            tc.tile_pool(name="zin", bufs=NBATCH) as zpool,
            tc.tile_pool(name="zn", bufs=4) as npool,
            tc.tile_pool(name="persist", bufs=1) as ppool,
            tc.tile_pool(name="scr", bufs=2) as spool,
            tc.tile_pool(name="pscr", bufs=2) as pspool,
            tc.tile_pool(name="psum", bufs=2, space=bass.MemorySpace.PSUM) as qpool,
        ):
            negI_sb = cpool.tile([128, 128], f32)
            ones_sb = cpool.tile([128, 1], f32)

            ssq = ppool.tile([128, NBATCH * TPB], f32)
            lnssq = ppool.tile([128, NBATCH * TPB], f32)
            inv = ppool.tile([128, NBATCH * TPB], f32)
            znT = ppool.tile([128, N], bf16)
            # accum slots: 0..3 = q0 512-wide warmup pieces (r=0 only),
            # 4..6 = chunks q=1..3; unused slots are memset to 0.
            sexp = ppool.tile([128, RB, QB + 3], f32)
            pos = ppool.tile([128, RB], f32)

            # Input DMAs: batch 0's four sub-DMAs go first on the idle SP
            # queue (565ns issue each); batches 1-3 + consts on gpsimd.
            zin_tiles = [
                zpool.tile([128, TPB, 128], f32, name=f"zin{b}")
                for b in range(NBATCH)
            ]

            def zin_dma(eng, b, s):
                r0 = 2048 * b + 512 * s
                src = z_dram[r0 : r0 + 512, :].rearrange("(t p) d -> p t d", p=128)
                eng.dma_start(zin_tiles[b][:, 4 * s : 4 * s + 4, :], src)

            # batch 0 first, alone on the wire (SP queue); batches 1-3 are
            # gated behind batch 0's arrival via a dummy gpsimd read so they
            # don't steal DMA bandwidth from the critical prologue chain.
            for s in range(4):
                zin_dma(nc.sync, 0, s)
            gate = cpool.tile([128, 4], f32)
            nc.gpsimd.tensor_copy(gate[:], zin_tiles[0][:, 15, 0:4])
            for b in range(1, NBATCH):
                for s in range(4):
                    zin_dma(nc.gpsimd, b, s)
            nc.gpsimd.dma_start(negI_sb[:], negI_dram[:])
            nc.gpsimd.dma_start(ones_sb[:], ones_dram[:])

            scr_g = {}

            def ssq_mul(g):
                b, s = g // 4, g % 4
                scr = spool.tile([128, TPG * 128], f32, tag="sq")
                scr_g[g] = scr
                zv = zin_tiles[b][:, 4 * s : 4 * s + 4, :].rearrange(
                    "p t d -> p (t d)"
                )
                nc.vector.tensor_mul(scr[:], zv, zv)

            def ssq_red(g):
                b, s = g // 4, g % 4
                j0 = TPB * b + TPG * s
                nc.vector.reduce_sum(
                    ssq[:, j0 : j0 + TPG],
                    scr_g.pop(g)[:].rearrange("p (t d) -> p t d", d=128),
                    axis=AX.X,
                )

            def norms(b, g0=None):
                # 1/||z|| = exp(-0.5*ln(ssq)); stays in the Ln/Exp ACT table.
                # g0 set: per-group [128,4] norms (batch-0 pipelining).
                if g0 is None:
                    j0, w = TPB * b, TPB
                else:
                    j0, w = TPG * g0, TPG
                nc.scalar.activation(
                    lnssq[:, j0 : j0 + w], ssq[:, j0 : j0 + w], AF.Ln
                )
                nc.scalar.activation(
                    inv[:, j0 : j0 + w], lnssq[:, j0 : j0 + w],
                    AF.Exp, scale=-0.5,
                )

            zn_tiles = {}

            def tsm(g):
                b, s = g // 4, g % 4
                zn = zn_tiles[g] = npool.tile(
                    [128, TPG, 128], bf16, name=f"zn{g}", tag="zn"
                )
                for t in range(4):
                    j = TPB * b + 4 * s + t
                    nc.vector.tensor_scalar_mul(
                        zn[:, t, :],
                        zin_tiles[b][:, 4 * s + t, :],
                        inv[:, j : j + 1],
                    )

            def build_trans(g, eng=None):
                zn = zn_tiles.pop(g)
                c0 = 512 * g
                (eng or nc.sync).dma_start_transpose(
                    znT[:, c0 : c0 + 512].rearrange("p (t c) -> p t c", c=128),
                    zn[:].rearrange("p t d -> p (t d)"),
                )

            def pos_dot(r):
                # pos_r = 10 * cos(row_r, row_r+4096) from raw zin tiles:
                # dot on zin, then scale by the two inverse norms.
                scr = pspool.tile([128, 128], f32, tag="pd")
                nc.vector.tensor_mul(
                    scr[:], zin_tiles[0][:, r, :], zin_tiles[2][:, r, :]
                )
                dot = pspool.tile([128, 1], f32, tag="pe")
                nc.vector.reduce_sum(dot[:], scr[:], axis=AX.X)
                nc.vector.tensor_mul(dot[:], dot[:], inv[:, r : r + 1])
                nc.vector.tensor_mul(
                    pos[:, r : r + 1], dot[:], inv[:, 32 + r : 33 + r]
                )

            # --- prologue: batch 0 only ---
            # DVE runs the 4 muls/reds back-to-back (per-group norms overlap
            # on ACT); tsm+xbar chains follow per group.
            nc.vector.memset(sexp[:], 0.0)
            for g in range(4):
                ssq_mul(g)
                ssq_red(g)
                norms(0, g0=g)
            for g in range(4):
                tsm(g)
                # alternate prologue xbars between ACT (idle until the exp
                # stream starts) and SP so they land pairwise in parallel.
                build_trans(g, eng=nc.scalar if g % 2 == 0 else nc.sync)

            # --- main loop: q-outer, r-inner; build batch q+1 under chunk q ---
            for q in range(QB):
                b = q + 1
                for r in range(RB):
                    lhsT = znT[:, 128 * r : 128 * (r + 1)]
                    ps = qpool.tile([128, 2048], f32, tag="mm")
                    for k in range(KB):
                        c0 = 2048 * q + 512 * k
                        nc.tensor.matmul(
                            ps[:, 512 * k : 512 * (k + 1)],
                            lhsT,
                            znT[:, c0 : c0 + 512],
                            start=True,
                            stop=True,
                        )
                    if q == 0:
                        sub = ps[:, 128 * r : 128 * (r + 1)]
                        nc.vector.tensor_add(sub, sub, negI_sb[:])
                    if q == 0 and r == 0:
                        # warmup: 512-wide exps so ACT starts as soon as the
                        # first znT group lands, not after all four.
                        for k in range(KB):
                            nc.scalar.activation(
                                ps[:, 512 * k : 512 * (k + 1)],
                                ps[:, 512 * k : 512 * (k + 1)],
                                AF.Exp,
                                scale=INV_T,
                                accum_out=sexp[:, 0, k : k + 1],
                            )
                    else:
                        nc.scalar.activation(
                            ps[:],
                            ps[:],
                            AF.Exp,
                            scale=INV_T,
                            accum_out=sexp[:, r, 3 + q : 4 + q],
                        )
                    if b < NBATCH:
                        g = 4 * b + (r % 4)
                        if r < 4:
                            ssq_mul(g)
                            ssq_red(g)
                        else:
                            if r == 4:
                                norms(b)
                            tsm(g)
                            build_trans(g)
                    elif q == QB - 1:
                        pos_dot(r)

            # --- epilogue ---
            s8 = ppool.tile([128, RB], f32)
            nc.vector.reduce_sum(s8[:], sexp[:], axis=AX.X)
            lse = ppool.tile([128, RB], f32)
            nc.scalar.activation(lse[:], s8[:], AF.Ln)
            poss = ppool.tile([128, RB], f32)
            nc.scalar.mul(poss[:], pos[:], INV_T)
            acc = ppool.tile([128, RB], f32)
            nc.vector.tensor_sub(acc[:], lse[:], poss[:])
            tot = ppool.tile([128, 1], f32)
            nc.vector.reduce_sum(tot[:], acc[:], axis=AX.X)
            psf = qpool.tile([128, 2048], f32, tag="mm")
            nc.tensor.matmul(
                psf[0:1, 0:1], ones_sb[:], tot[:], start=True, stop=True
            )
            res = ppool.tile([1, 1], f32)
            nc.scalar.mul(res[:], psf[0:1, 0:1], 1.0 / N)
            nc.gpsimd.dma_start(loss_dram[:], res[:])

    nc.compile()
    return nc


def get_nc():
    if "nc" not in _cache:
        _cache["nc"] = build()
    return _cache["nc"]


def make_in_maps(z_i: np.ndarray, z_j: np.ndarray):
    z = np.concatenate(
        [np.asarray(z_i, np.float32), np.asarray(z_j, np.float32)], axis=0
    )
    return [
        {"z_roll": np.ascontiguousarray(np.roll(z, -RPC * c, axis=0))}
        for c in range(N_CORES)
    ]


def kernel(**inputs) -> np.ndarray:
    in_maps = make_in_maps(inputs["z_i"], inputs["z_j"])
    nc = get_nc()
    res = run_bass_kernel_spmd(nc, in_maps, list(range(N_CORES)))
    kernel.last_results = res
    total = np.float32(0.0)
    for r in res.results:
        total = np.float32(total + np.float32(np.asarray(r["loss_part"]).reshape(())))
    return np.float32(total)
